# revision 33
# baseline (speedup 1.0000x reference)
"""ArHmmLm kernel for 8 TRN2 NeuronCores.

Device work (the memory roofline): the (252,256)@(256,524288) logit
matmul + exp + vocab-axis sum, vocab-sharded 8 ways.  proj_W is
quantized to fp8e4 on the host (16MB per core, streamed once) and the
matmul runs in DoubleRow perf mode (K=256 contracted per pass).  The
vocab-sum of exp(logit) is split across engines working on disjoint
(m-tile, c) units of the PSUM output:
  - "act" units:  exact exp via activation(Exp, scale=1/g) + accum
  - "dve" units:  deg-3 Taylor of exp(x/g)-1 via a custom DVE op
                  (cubic Horner + add-reduction in one instruction)
Logits are ~N(0, 0.04): the x^4/24 truncation error of the deg-3 units
is ~1e-8 relative on each vocab-sum - far inside the 2e-2 gate.

Host glue (all tiny, <0.2% of FLOPs): embedding gather, the conv/MLP
head, start/transition heads, the gather of observed-token logits in
exact f32, log(S) and the C=64 forward-backward scan for
evidence/elbo.
"""
import numpy as np

B, T, V, C, H = 4, 64, 8192, 64, 256
NCORES = 8
VS = V // NCORES           # 1024 vocab per core
M = B * (T - 1)            # 252 logit rows
MP = 256                   # padded rows (2 psum tiles of 128)
AH = 64.0                  # fp8 scale for h
AW = 64.0                  # fp8 scale for w
GAM = AH * AW              # logit scale in psum: x = GAM * logit

# unit counts per consumer type (sum = 2*C = 128)
_COUNTS = (("act", 62), ("dve", 66))


def _wrr_assign(n):
    """Smooth weighted round-robin over _COUNTS quotas."""
    total = sum(c for _, c in _COUNTS)
    assert total == n
    cred = {k: 0.0 for k, _ in _COUNTS}
    seq = []
    for _ in range(n):
        for k, c in _COUNTS:
            cred[k] += c
        e = max(cred, key=lambda k: cred[k])
        cred[e] -= total
        seq.append(e)
    return seq


# engine per (c, mt) unit, interleaved along the production order
_PROD = [(c, mt) for c in range(C) for mt in range(2)]
_ASSIGN = dict(zip(_PROD, _wrr_assign(2 * C)))
_ENG_IDX = {"act": 0, "dve": 1}

_GRAPH = None
LAST_EXEC_NS = None
TRACE = False

WBUFS = 20


def _register_exp3_op():
    """Register a custom DVE op computing the cubic Horner polynomial
    ((x*c3 + c2)*x + c1)*x with a free-axis add-reduction to accum_out.
    With c_k = 1/(k! * GAM^k) this is GAM-descaled deg-3 Taylor of
    exp(x/GAM) - 1; the vocab-sum of the missing constant 1 is added on
    the host.  Single Src0 stream, so PSUM input is legal (unlike
    scalar_tensor_tensor, which would need two PSUM reads)."""
    from operator import add as _opadd
    from concourse import dve_ops
    from concourse.dve_spec import Spec, Src0, C0, C1, C2, Zero, lower
    from concourse.dve_uop import DveOpSpec

    name = "EXP3_REDUCE_ANT"
    for op in dve_ops.OPS:
        if op.name == name:
            return op

    body = ((Src0 * C0 + C1) * Src0 + C2) * Src0

    def _ref(in0, in1, s0, s1, imm2):
        x = in0.astype(np.float32)
        b = (((x * s0 + s1) * x + imm2) * x).astype(np.float32)
        return b, b.reshape(b.shape[0], -1).sum(axis=-1, keepdims=True)

    spec = Spec(body=body, accum=_opadd, accum_init=Zero, reference=_ref)
    row = dve_ops._CUSTOM_DVE_ROW_BASE + len(dve_ops.OPS)
    dve_ops._SUB_OPCODE_FOR_NAME[name] = row
    shas = {}
    for ver in ("v3", "v4"):
        u = lower(spec, ver=ver)
        shas[ver] = DveOpSpec(name=name, opcode=row, uops=u,
                              rd1_en=False).sha(ver)
    op = dve_ops.DveOp(name, spec, subdim=False, uops_sha=shas)
    dve_ops.OPS.append(op)
    dve_ops.CUSTOM_DVE_SPECS[name] = spec
    return op


def _build_graph():
    import concourse.bass as bass
    import concourse.mybir as mybir
    import concourse.tile as tile
    from concourse import bacc
    from concourse.tile_rust import add_dep_helper

    f32 = mybir.dt.float32
    bf16 = mybir.dt.bfloat16
    f8 = mybir.dt.float8e4
    DR = mybir.MatmulPerfMode.DoubleRow
    exp3 = _register_exp3_op()
    # GAM-descaled Taylor coefficients for the DVE units
    c1, c2, c3 = 1.0 / GAM, 0.5 / GAM**2, 1.0 / 6.0 / GAM**3
    nc = bacc.Bacc("TRN2", target_bir_lowering=False, debug=False,
                   num_devices=NCORES)
    # hT packed: [128, k(2) * 256]; col = k*256 + (mt*128 + m)
    hT_ext = nc.declare_dram_parameter("hT", [128, 2 * MP], f8, isOutput=False)
    # w packed: [128, c(64) * k(2) * n(1024)]; col = c*2048 + k*1024 + n
    w_ext = nc.declare_dram_parameter("w", [128, C * 2 * VS], f8,
                                      isOutput=False)
    # stage rows: (eng*2 + mt)*128 + p
    out_ext = nc.declare_dram_parameter("out", [4 * 128, C], f32,
                                        isOutput=True)

    with tile.TileContext(nc) as tc:
        with (
            tc.tile_pool(name="h", bufs=1) as hpool,
            tc.tile_pool(name="w", bufs=WBUFS) as wpool,
            tc.tile_pool(name="scr", bufs=1) as spool,
            tc.tile_pool(name="psum", bufs=4, space="PSUM") as ppool,
        ):
            # hT flat layout: col = mt*256 + k*128 + m  (per-mt [k,m] block
            # contiguous so the DoubleRow stationary AP is a linear run)
            ht = hpool.tile([128, 2 * MP], f8, tag="ht", name="ht")
            nc.sync.dma_start(ht[:], hT_ext[:, :])
            lhs = [ht[:, mt * 256:(mt + 1) * 256].rearrange(
                       "p (k m) -> p k m", k=2) for mt in range(2)]
            # PE-side absorber for the ht DMA wait
            hdum = nc.tensor.ldweights(ht[:, 0:1])

            # per-engine scratch (elementwise outputs, values unused)
            scr = {e: spool.tile([128, VS], bf16, tag=f"scr_{e}",
                                 name=f"scr_{e}")
                   for e in ("act", "dve")}
            # per (engine, mt) accum stages
            stage = {(e, mt): spool.tile([128, C], f32, tag=f"st_{e}{mt}",
                                         name=f"st_{e}{mt}")
                     for e in ("act", "dve") for mt in range(2)}

            prev_dum = hdum
            for c in range(C):
                # w flat layout per c-chunk: col = q*1024 + k*512 + n; each
                # matmul's [k, n] moving block is a contiguous 1024B run.
                wt = wpool.tile([128, 2 * VS], f8, tag="wt", name="wt")
                lo = c * 2 * VS
                nc.sync.dma_start(wt[:], w_ext[:, lo:lo + 2 * VS])
                # absorber for this c-group's DMA wait
                gdum = nc.tensor.ldweights(wt[:, 0:1])
                add_dep_helper(gdum.ins, prev_dum.ins, sync=False,
                               reason="keep absorbers in PE order")
                prev_dum = gdum
                for mt in range(2):
                    ps = ppool.tile([128, VS], f32, tag="ps", name="ps")
                    for q in range(2):
                        rhs = wt[:, q * VS:(q + 1) * VS].rearrange(
                            "p (k n) -> p k n", k=2)
                        mm = nc.tensor.matmul(
                            ps[:, q * 512:(q + 1) * 512],
                            lhs[mt], rhs,
                            start=True, stop=True, perf_mode=DR)
                        if mt == 0 and q == 0:
                            add_dep_helper(mm.ins, prev_dum.ins, sync=False,
                                           reason="matmuls after absorber")
                    eng = _ASSIGN[(c, mt)]
                    acc = stage[(eng, mt)][:, c:c + 1]
                    if eng == "act":
                        nc.scalar.activation(
                            scr["act"][:], ps[:],
                            mybir.ActivationFunctionType.Exp,
                            scale=1.0 / GAM, accum_out=acc)
                    elif eng == "dve":
                        nc.vector._custom_dve(
                            exp3, out=scr["dve"][:], in0=ps[:],
                            s0=c3, s1=c2, imm2=c1, accum_out=acc)

            for e in ("act", "dve"):
                for mt in range(2):
                    r0 = (_ENG_IDX[e] * 2 + mt) * 128
                    nc.sync.dma_start(out_ext[r0:r0 + 128, :],
                                      stage[(e, mt)][:])
    if not nc.is_finalized():
        nc.finalize()
    return nc


def _relu(x):
    return np.maximum(x, 0.0)


def _residual(x, W1, b1, W2, b2):
    return _relu(_relu(x @ W1 + b1) @ W2 + b2) + x


def _log_softmax(x, axis=-1):
    m = np.max(x, axis=axis, keepdims=True)
    s = np.log(np.sum(np.exp(x - m), axis=axis, keepdims=True))
    return x - m - s


def _softmax(x, axis=-1):
    m = np.max(x, axis=axis, keepdims=True)
    e = np.exp(x - m)
    return e / np.sum(e, axis=axis, keepdims=True)


def _lse(x, axis=-1):
    m = np.max(x, axis=axis)
    return m + np.log(np.sum(np.exp(x - np.expand_dims(m, axis)), axis=axis))


def kernel(**inputs):
    global _GRAPH, LAST_EXEC_NS
    import ml_dtypes
    from concourse.bass_utils import run_bass_kernel_spmd

    f8np = ml_dtypes.float8_e4m3

    text = np.asarray(inputs["text"])
    lengths = np.asarray(inputs["lengths"])
    f = {k: np.asarray(v, dtype=np.float32) for k, v in inputs.items()
         if k not in ("text", "lengths")}

    # ---- host: h = conv+MLP features (252,256); 0.15% of total FLOPs
    x = np.concatenate([np.zeros((B, 1), text.dtype), text[:, :-1]], axis=1)
    e = f["emb_W"][x]                                            # (B,T,H)
    h = _relu(e[:, :-1] @ f["conv_W0"] + e[:, 1:] @ f["conv_W1"] + f["conv_b"])
    h = _residual(h, f["mW1"], f["mb1"], f["mW2"], f["mb2"])     # (B,T-1,H)
    hm = h.reshape(M, H).astype(np.float32)

    # ---- host: start / transition heads (C=64, tiny)
    start = _log_softmax(
        _residual(f["start_emb"], f["sW1"], f["sb1"], f["sW2"], f["sb2"])
        @ f["s_out_W"] + f["s_out_b"])                           # (C,)
    transition = _log_softmax(
        _residual(f["state_emb"], f["tW1"], f["tb1"], f["tW2"], f["tb2"])
        @ f["t_out_W"] + f["t_out_b"], axis=-1).T                # (C_next, C_prev)

    # ---- host: observed-token logits in exact f32 (252 rows, 8 MFLOP)
    obs = text[:, 1:].reshape(M)
    Wobs = f["proj_W"].reshape(V, C, H)[obs]                     # (M,C,H)
    obs_logits = np.einsum("mh,mch->mc", hm, Wobs)               # (M,C)

    # ---- device: vocab-sharded partial sum(exp(logits)) over v
    if _GRAPH is None:
        _GRAPH = _build_graph()
    hT = np.zeros((H, MP), np.float32)
    hT[:, :M] = hm.T * AH
    # h col = mt*256 + k*128 + m'
    hq = np.ascontiguousarray(
        hT.reshape(2, 128, 2, 128).transpose(1, 2, 0, 3)).reshape(128, 2 * MP)
    hq = hq.astype(f8np)
    # w col = (c*2+q)*1024 + k*512 + n
    #       = proj_W[(s*VS + q*512 + n)*C + c, k*128+p] * AW
    A = (f["proj_W"] * AW).reshape(NCORES, 2, 512, C, 2, 128)    # s,q,n,c,k,p
    A = np.ascontiguousarray(A.transpose(0, 5, 3, 1, 4, 2))      # s,p,c,q,k,n
    Wq = A.astype(f8np).reshape(NCORES, 128, C * 2 * VS)
    in_maps = [{"hT": hq, "w": Wq[i]} for i in range(NCORES)]
    res = run_bass_kernel_spmd(_GRAPH, in_maps, core_ids=list(range(NCORES)),
                               trace=TRACE)
    LAST_EXEC_NS = res.exec_time_ns

    # combine per-core stages into the full-vocab S[m, c] = sum_v exp(logit)
    S = np.zeros((MP, C), np.float64)
    for r in res.results:
        o = r["out"].astype(np.float64)                          # (512, C)
        for mt in range(2):
            rows = slice(mt * 128, (mt + 1) * 128)
            for c in range(C):
                eng = _ASSIGN[(c, mt)]
                blk = (_ENG_IDX[eng] * 2 + mt) * 128
                col = o[blk:blk + 128, c]
                if eng == "dve":
                    S[rows, c] += VS + col     # Taylor constant term
                else:
                    S[rows, c] += col
    # ---- host: em, potentials, forward scan, marginals, elbo (C=64, tiny)
    em = (obs_logits.astype(np.float64) - np.log(S[:M])).reshape(B, T - 1, C)
    pot = transition[None, None].astype(np.float64) + em[:, :, :, None]
    pot[:, 0] += start[None, :]                                  # over prev axis

    alphas = np.zeros((T - 1, B, C))
    alphas[0] = _lse(pot[:, 0], axis=-1)
    for t in range(1, T - 1):
        alphas[t] = _lse(pot[:, t] + alphas[t - 1][:, None, :], axis=-1)
    idx = np.clip(lengths - 2, 0, T - 2)
    final = alphas[idx, np.arange(B)]                            # (B,C)
    evidence = _lse(final, axis=-1).sum()

    marg = np.zeros_like(pot)                                    # (B,T-1,C,C)
    for b in range(B):
        L = int(idx[b])
        g = _softmax(final[b])                                   # d logZ/d alpha_L
        for t in range(L, 0, -1):
            w = _softmax(pot[b, t] + alphas[t - 1][b][None, :], axis=-1)
            marg[b, t] = g[:, None] * w
            g = (g[:, None] * w).sum(axis=0)
        marg[b, 0] = _softmax(pot[b, 0], axis=-1) * g[:, None]
    mask = (np.arange(T)[None, :] < lengths[:, None])[:, 1:]
    elbo = (marg * pot * mask[:, :, None, None]).sum()

    return np.stack([elbo, evidence]).astype(np.float32)
